# revision 7
# baseline (speedup 1.0000x reference)
"""Trainium2 Bass kernel for multi-head cross-attention.

Reference computation (fp32):
  q = x @ Wq; k = ctx @ Wk; v = ctx @ Wv              (per batch)
  sim = einsum('bihd,bjhd->bhij', q, k) * 1/sqrt(64)
  out = softmax(sim) @ v ; out = out @ Wo + bo

Shapes: x (4, 2048, 1024), context (4, 2048, 768), HEADS=8, DIM_HEAD=64.

Sharding: 8 cores = (batch b = core//2) x (query half = core%2). Each core
computes the full attention for its 1024 query rows across all 8 heads with
replicated weights; outputs concatenate — no cross-core reduction.

On-core dataflow. Matmul operands are bf16 with fp32 PSUM accumulation:
  - x^T and ctx^T are prepared host-side (feature dim on partitions),
    pre-cast to bf16 on host along with the weights.
  - q^T[c,i], k^T[c,j]  via lhsT=W, rhs=x^T/ctx^T   (feature-major outputs)
  - v[j,c]              via lhsT=ctx^T, rhs=Wv       (context-major output),
    stored per head with an extra ones column: [v_h | 1] (65 cols/head)
  - attention runs head-PAIR by head-pair (heads 2t, 2t+1 live on SBUF
    partitions 0-63 / 64-127 of feature tile t).  The K=64 score matmuls
    for the two heads are issued back-to-back with explicit tile_position
    (0,0)/(64,0) so the PE runs them CONCURRENTLY on disjoint row groups.
    PV likewise splits its K=128 contraction into two concurrent K=64
    row-tiles accumulating into separate PSUM banks (summed on DVE at the
    end of the pair).  Queries are processed in halves of 512 so all of
    this fits in the 8 PSUM banks:
      S psum: 2 tiles [128, 2x512] (2 context blocks x 512 queries)
      PV:     4 accumulators [65, 512] (2 heads x 2 K-halves)
    Steady state: ACT (exp, the bottleneck of this phase) stays 100% busy;
    the PE's score+PV work for a round fits inside one exp period.
  - normalization per (head, query-half): O' halves summed on DVE,
    reciprocal of the denominator row via the fast approx DVE op, a
    lane-shift DMA to partition 0, gpsimd partition_broadcast, DVE mult.
    (All off the PE so its queue never stalls — a PE-visible gap
    re-throttles the HAM clock gate to half speed.)
  - final: F = O^T.T @ Wo + ones^T @ bo (bias via K=1 matmul into the same
    PSUM accumulation group).

Input DMAs are split per 128-row tile so the first projection matmuls
start as soon as their operands land rather than after the full tensor;
Wo/bo (only needed by the output projection) are fetched after the
projection-phase operands.
"""

import ml_dtypes
import numpy as np

import concourse.bass as bass
import concourse.tile as tile
from concourse import bacc, mybir
from concourse.bass_utils import run_bass_kernel_spmd

F32 = mybir.dt.float32
BF16 = mybir.dt.bfloat16

B = 4
NQ_FULL = 2048
NQ = 1024  # local query rows per core
NC = 2048
DQ = 1024
DC = 768
H = 8
DH = 64
INNER = H * DH  # 512
SCALE = DH ** -0.5

AT = DQ // 128   # 8  k-tiles of the q-projection contraction
BT = DC // 128   # 6  k-tiles of the k/v-projection contraction
CT = INNER // 128  # 4 feature tiles of q^T/k^T/o^T
IB = NQ // 128   # 8  query-row blocks
JB = NC // 128   # 16 context-row blocks

_CACHE = {}


def _build_program():
    nc = bacc.Bacc(
        "TRN2",
        target_bir_lowering=False,
        debug=False,
        enable_asserts=False,
    )

    xT = nc.dram_tensor("xT", [DQ, NQ], BF16, kind="ExternalInput").ap()
    ctxT = nc.dram_tensor("ctxT", [DC, NC], BF16, kind="ExternalInput").ap()
    wq = nc.dram_tensor("Wq", [DQ, INNER], BF16, kind="ExternalInput").ap()
    wk = nc.dram_tensor("Wk", [DC, INNER], BF16, kind="ExternalInput").ap()
    wv = nc.dram_tensor("Wv", [DC, INNER], BF16, kind="ExternalInput").ap()
    wo = nc.dram_tensor("Wo", [INNER, DQ], BF16, kind="ExternalInput").ap()
    bo = nc.dram_tensor("bo", [DQ], BF16, kind="ExternalInput").ap()
    out = nc.dram_tensor("out", [NQ, DQ], F32, kind="ExternalOutput").ap()

    with tile.TileContext(nc) as tc:
        with nc.allow_low_precision(reason="bf16 matmul operands"):
            _emit(nc, tc, xT, ctxT, wq, wk, wv, wo, bo, out)

    nc.compile()
    return nc


def _emit(nc, tc, xT, ctxT, wq, wk, wv, wo, bo, out):
    from contextlib import ExitStack

    with ExitStack() as ctx:
        const = ctx.enter_context(tc.tile_pool(name="const", bufs=1))
        persist = ctx.enter_context(tc.tile_pool(name="persist", bufs=1))
        expp = ctx.enter_context(tc.tile_pool(name="expp", bufs=4))
        opool = ctx.enter_context(tc.tile_pool(name="opool", bufs=4))
        rpool = ctx.enter_context(tc.tile_pool(name="rpool", bufs=4))
        otmp = ctx.enter_context(tc.tile_pool(name="otmp", bufs=2))
        outp = ctx.enter_context(tc.tile_pool(name="outp", bufs=2))

        # --- persistent feature-major activations ---
        qT_sb = persist.tile([128, CT, NQ], BF16)
        kT_sb = persist.tile([128, CT, NC], BF16)
        v_sb = persist.tile([128, JB, H * 65], BF16)  # [v_h | 1] per head
        oT_sb = persist.tile([128, CT, NQ], BF16)
        wo_sb = persist.tile([128, CT, DQ], BF16)
        bo_sb = const.tile([1, DQ], BF16)
        onesF = const.tile([128, 128], F32)
        nc.vector.memset(onesF, 1.0)
        ones1 = const.tile([1, 128], BF16)  # bias-matmul lhsT
        nc.vector.tensor_copy(ones1, onesF[0:1, :])

        v4 = v_sb.rearrange("p j (h e) -> p j h e", e=65)
        for jb in range(JB):
            nc.vector.tensor_copy(v4[:, jb, :, 64:65], onesF[:, 0:H].unsqueeze(-1))

        # --- phase A: q^T = (x @ Wq)^T via lhsT=Wq, rhs=x^T ---
        with tc.tile_pool(name="ps_pj", bufs=2, space="PSUM") as ps_pj:
            with tc.tile_pool(name="phA", bufs=1) as phA:
                xT_sb = phA.tile([128, AT, NQ], BF16)
                wq_sb = phA.tile([128, AT, INNER], BF16)
                xTr = xT.rearrange("(t p) i -> p t i", p=128)
                wqr = wq.rearrange("(t p) c -> p t c", p=128)
                for a in range(AT):
                    nc.sync.dma_start(out=wq_sb[:, a, :], in_=wqr[:, a, :])
                    nc.sync.dma_start(out=xT_sb[:, a, :], in_=xTr[:, a, :])
                for t in range(CT):
                    ps = ps_pj.tile([128, NQ], F32, tag="pa")
                    for a in range(AT):
                        for ch in range(2):
                            nc.tensor.matmul(
                                ps[:, ch * 512:(ch + 1) * 512],
                                lhsT=wq_sb[:, a, t * 128:(t + 1) * 128],
                                rhs=xT_sb[:, a, ch * 512:(ch + 1) * 512],
                                start=(a == 0),
                                stop=(a == AT - 1),
                            )
                    nc.vector.tensor_copy(qT_sb[:, t, :], ps)

            # --- phase B: k^T and v from streamed ctx^T quarters ---
            with tc.tile_pool(name="phBw", bufs=1) as phBw:
                wk_sb = phBw.tile([128, BT, INNER], BF16)
                wv_sb = phBw.tile([128, BT, INNER], BF16)
                wkr = wk.rearrange("(t p) c -> p t c", p=128)
                wvr = wv.rearrange("(t p) c -> p t c", p=128)
                for b in range(BT):
                    nc.sync.dma_start(out=wk_sb[:, b, :], in_=wkr[:, b, :])
                    nc.sync.dma_start(out=wv_sb[:, b, :], in_=wvr[:, b, :])
                ctxTr = ctxT.rearrange("(t p) j -> p t j", p=128)
                # Wo/bo only matter for the output projection; queue their
                # DMAs behind the projection-critical loads.
                wor = wo.rearrange("(t p) e -> p t e", p=128)
                for t in range(CT):
                    nc.sync.dma_start(out=wo_sb[:, t, :], in_=wor[:, t, :])
                nc.sync.dma_start(out=bo_sb, in_=bo.unsqueeze(0))
                with tc.tile_pool(name="phBx", bufs=2) as phBx:
                    for jq in range(4):
                        cx = phBx.tile([128, BT, 512], BF16, tag="cx")
                        for b in range(BT):
                            nc.sync.dma_start(
                                out=cx[:, b, :],
                                in_=ctxTr[:, b, jq * 512:(jq + 1) * 512],
                            )
                        for t in range(CT):
                            ps = ps_pj.tile([128, NQ], F32, tag="pa")
                            for b in range(BT):
                                nc.tensor.matmul(
                                    ps[:, 0:512],
                                    lhsT=wk_sb[:, b, t * 128:(t + 1) * 128],
                                    rhs=cx[:, b, :],
                                    start=(b == 0),
                                    stop=(b == BT - 1),
                                )
                            nc.vector.tensor_copy(
                                kT_sb[:, t, jq * 512:(jq + 1) * 512], ps[:, 0:512]
                            )
                        for q in range(4):
                            jb = jq * 4 + q
                            ps = ps_pj.tile([128, NQ], F32, tag="pa")
                            for b in range(BT):
                                nc.tensor.matmul(
                                    ps[:, 0:512],
                                    lhsT=cx[:, b, q * 128:(q + 1) * 128],
                                    rhs=wv_sb[:, b, :],
                                    start=(b == 0),
                                    stop=(b == BT - 1),
                                )
                            nc.vector.tensor_copy(
                                v4[:, jb, :, 0:64],
                                ps[:, 0:512].rearrange("p (h d) -> p h d", d=DH),
                            )

        # --- attention: head pairs, concurrent PE row-tiles ---
        with tc.tile_pool(name="ps_s", bufs=1, space="PSUM") as ps_s, \
             tc.tile_pool(name="ps_pv", bufs=1, space="PSUM") as ps_pv:
            for qh in range(2):
                q0 = qh * 512
                for t in range(CT):
                    he, ho = 2 * t, 2 * t + 1
                    # PV accumulators: one [65, 512] per head (serial K=128)
                    opv = [
                        ps_pv.tile([65, 512], F32, tag=f"pv{h}",
                                   name=f"opv{h}")
                        for h in range(2)
                    ]
                    es_tiles = []
                    for jbp in range(8):
                        sps = []
                        ess = []
                        for hh, (po, tp) in enumerate(((0, (0, 0)), (64, (64, 0)))):
                            sp = ps_s.tile([128, 1024], F32, tag=f"s{hh}")
                            # scores for 2 context blocks x 512 queries
                            for jj in range(2):
                                jb = jbp * 2 + jj
                                nc.tensor.matmul(
                                    sp[:, jj * 512:(jj + 1) * 512],
                                    lhsT=kT_sb[po:po + 64, t,
                                               jb * 128:(jb + 1) * 128],
                                    rhs=qT_sb[po:po + 64, t, q0:q0 + 512],
                                    start=True,
                                    stop=True,
                                    tile_position=tp,
                                )
                            # PV of the PREVIOUS round goes on the PE queue
                            # here so it overlaps this round's exp drain.
                            if jbp > 0:
                                self_h, prev_es = hh, es_tiles[jbp - 1][hh]
                                for jj in range(2):
                                    jb = (jbp - 1) * 2 + jj
                                    nc.tensor.matmul(
                                        opv[self_h][:, :],
                                        lhsT=v4[:, jb, he + self_h, :],
                                        rhs=prev_es[:,
                                                    jj * 512:(jj + 1) * 512],
                                        start=(jbp == 1 and jj == 0),
                                        stop=False,
                                    )
                            es = expp.tile([128, 1024], BF16, tag=f"es{hh}")
                            nc.scalar.activation(
                                es, sp, mybir.ActivationFunctionType.Exp,
                                scale=SCALE,
                            )
                            sps.append(sp)
                            ess.append(es)
                        es_tiles.append(ess)
                    # drain: PV for the last round (jbp=7)
                    for hh in range(2):
                        for jj in range(2):
                            jb = 7 * 2 + jj
                            nc.tensor.matmul(
                                opv[hh][:, :],
                                lhsT=v4[:, jb, he + hh, :],
                                rhs=es_tiles[7][hh][:,
                                                    jj * 512:(jj + 1) * 512],
                                start=False,
                                stop=(jj == 1),
                            )
                    # normalize the pair (odd head first: its result takes an
                    # extra SBUF->SBUF DMA hop to partitions 64-127).
                    for hh in (1, 0):
                        osum = opool.tile([65, 512], F32, tag=f"os{hh}")
                        nc.vector.tensor_copy(osum, opv[hh])
                        rt = rpool.tile([65, 512], F32, tag=f"rt{hh}")
                        nc.vector.reciprocal(rt[64:65, :], osum[64:65, :])
                        r0 = rpool.tile([1, 512], F32, tag=f"r0{hh}")
                        nc.sync.dma_start(out=r0, in_=rt[64:65, :])
                        rbx = rpool.tile([64, 512], F32, tag=f"rb{hh}")
                        nc.gpsimd.partition_broadcast(rbx, r0)
                        if hh == 0:
                            nc.vector.tensor_mul(
                                oT_sb[0:64, t, q0:q0 + 512],
                                osum[0:64, :], rbx,
                            )
                        else:
                            ot = otmp.tile([64, 512], BF16, tag="ot")
                            nc.vector.tensor_mul(ot, osum[0:64, :], rbx)
                            nc.sync.dma_start(
                                out=oT_sb[64:128, t, q0:q0 + 512], in_=ot
                            )

        # --- output projection: F = O^T.T @ Wo + bias ---
        with tc.tile_pool(name="ps_f", bufs=2, space="PSUM") as ps_f:
            for ib in range(IB):
                fp = ps_f.tile([128, NQ], F32, tag="pf")
                for ch in range(2):
                    for t in range(CT):
                        nc.tensor.matmul(
                            fp[:, ch * 512:(ch + 1) * 512],
                            lhsT=oT_sb[:, t, ib * 128:(ib + 1) * 128],
                            rhs=wo_sb[:, t, ch * 512:(ch + 1) * 512],
                            start=(t == 0),
                            stop=False,
                        )
                    nc.tensor.matmul(
                        fp[:, ch * 512:(ch + 1) * 512],
                        lhsT=ones1,
                        rhs=bo_sb[0:1, ch * 512:(ch + 1) * 512],
                        start=False,
                        stop=True,
                    )
                ost = outp.tile([128, DQ], F32)
                nc.vector.tensor_copy(ost, fp)
                nc.sync.dma_start(out=out[ib * 128:(ib + 1) * 128, :], in_=ost)


def get_program():
    if "nc" not in _CACHE:
        _CACHE["nc"] = _build_program()
    return _CACHE["nc"]


def make_in_maps(x, context, Wq, Wk, Wv, Wo, bo):
    bf = ml_dtypes.bfloat16
    in_maps = []
    wq_b = np.asarray(Wq).astype(bf)
    wk_b = np.asarray(Wk).astype(bf)
    wv_b = np.asarray(Wv).astype(bf)
    wo_b = np.asarray(Wo).astype(bf)
    bo_b = np.asarray(bo).astype(bf)
    for c in range(8):
        b, half = c // 2, c % 2
        in_maps.append({
            "xT": np.ascontiguousarray(
                x[b, half * NQ:(half + 1) * NQ, :].T
            ).astype(bf),
            "ctxT": np.ascontiguousarray(context[b].T).astype(bf),
            "Wq": wq_b,
            "Wk": wk_b,
            "Wv": wv_b,
            "Wo": wo_b,
            "bo": bo_b,
        })
    return in_maps


def kernel(x, context, Wq, Wk, Wv, Wo, bo):
    nc = get_program()
    in_maps = make_in_maps(x, context, Wq, Wk, Wv, Wo, bo)
    res = run_bass_kernel_spmd(nc, in_maps, list(range(8)))
    out = np.empty((B, NQ_FULL, DQ), np.float32)
    for c in range(8):
        b, half = c // 2, c % 2
        out[b, half * NQ:(half + 1) * NQ, :] = res.results[c]["out"]
    return out


# revision 10
# speedup vs baseline: 1.3165x; 1.3165x over previous
"""Trainium2 Bass kernel for multi-head cross-attention.

Reference computation (fp32):
  q = x @ Wq; k = ctx @ Wk; v = ctx @ Wv              (per batch)
  sim = einsum('bihd,bjhd->bhij', q, k) * 1/sqrt(64)
  out = softmax(sim) @ v ; out = out @ Wo + bo

Shapes: x (4, 2048, 1024), context (4, 2048, 768), HEADS=8, DIM_HEAD=64.

Sharding: 8 cores = (batch b = core//2) x (query half = core%2). Each core
computes the full attention for its 1024 query rows across all 8 heads with
replicated weights; outputs concatenate — no cross-core reduction.

On-core dataflow. Matmul operands are bf16 with fp32 PSUM accumulation:
  - x^T and ctx^T are prepared host-side (feature dim on partitions),
    pre-cast to bf16 on host along with the weights.
  - DMAs are consolidated (dispatch costs ~600ns on the Sync queue per
    instruction regardless of size) and ordered so the projection-phase
    operands land first; Wo/bo queue behind them.
  - q^T via lhsT=Wq, rhs=x^T, contraction-chunk-major with all four
    feature-tile PSUM accumulation groups open so each x^T chunk is
    consumed the moment it lands.
  - k^T and v from streamed ctx^T quarters (feature-major k^T; v stored
    per head with an extra ones column: [v_h | 1], 65 cols/head).
  - attention runs head-PAIR by head-pair (heads 2t, 2t+1 live on SBUF
    partitions 0-63 / 64-127 of feature tile t).  The K=64 score matmuls
    for the two heads are issued with explicit tile_position (0,0)/(64,0)
    so the PE can run them concurrently on disjoint row groups.  Queries
    are processed in halves of 512: S psum tiles pack 2 context blocks x
    512 queries so each exp() instruction still covers 1024 elements/lane.
    exp on ACT is the phase bottleneck; the schedule keeps it saturated.
  - HAM filler: the PE's real work per exp period (~1.7us of 2.22us)
    leaves ~0.5us idle slices, which the HAM clock governor punishes by
    dropping the PE to half clock (measured: k=4 for most of the phase,
    which then makes the PE the bottleneck).  Small throwaway matmuls
    into a scratch PSUM bank fill the idle slices to hold k=8.
  - normalization per (head, query-half): the PV accumulator's
    denominator row DMAs to partition 0 (custom DVE ops silently write
    nothing at base partition != 0 on HW — reciprocal must run at
    partition 0), fast-approx reciprocal, gpsimd partition_broadcast,
    DVE multiply straight out of PSUM.  All off the PE.
  - final: F = O^T.T @ Wo + ones^T @ bo (bias via K=1 matmul into the
    same PSUM accumulation group).
"""

import ml_dtypes
import numpy as np

import concourse.bass as bass
import concourse.tile as tile
from concourse import bacc, mybir
from concourse.bass_utils import run_bass_kernel_spmd

F32 = mybir.dt.float32
BF16 = mybir.dt.bfloat16

B = 4
NQ_FULL = 2048
NQ = 1024  # local query rows per core
NC = 2048
DQ = 1024
DC = 768
H = 8
DH = 64
INNER = H * DH  # 512
SCALE = DH ** -0.5

AT = DQ // 128   # 8  k-tiles of the q-projection contraction
BT = DC // 128   # 6  k-tiles of the k/v-projection contraction
CT = INNER // 128  # 4 feature tiles of q^T/k^T/o^T
IB = NQ // 128   # 8  query-row blocks
JB = NC // 128   # 16 context-row blocks

_CACHE = {}


def _build_program():
    nc = bacc.Bacc(
        "TRN2",
        target_bir_lowering=False,
        debug=False,
        enable_asserts=False,
    )

    xT = nc.dram_tensor("xT", [DQ, NQ], BF16, kind="ExternalInput").ap()
    ctxT = nc.dram_tensor("ctxT", [DC, NC], BF16, kind="ExternalInput").ap()
    wq = nc.dram_tensor("Wq", [DQ, INNER], BF16, kind="ExternalInput").ap()
    wk = nc.dram_tensor("Wk", [DC, INNER], BF16, kind="ExternalInput").ap()
    wv = nc.dram_tensor("Wv", [DC, INNER], BF16, kind="ExternalInput").ap()
    wo = nc.dram_tensor("Wo", [INNER, DQ], BF16, kind="ExternalInput").ap()
    bo = nc.dram_tensor("bo", [DQ], BF16, kind="ExternalInput").ap()
    out = nc.dram_tensor("out", [NQ, DQ], F32, kind="ExternalOutput").ap()

    with tile.TileContext(nc) as tc:
        with nc.allow_low_precision(reason="bf16 matmul operands"):
            _emit(nc, tc, xT, ctxT, wq, wk, wv, wo, bo, out)

    nc.compile()
    return nc


def _emit(nc, tc, xT, ctxT, wq, wk, wv, wo, bo, out):
    from contextlib import ExitStack

    with ExitStack() as ctx:
        const = ctx.enter_context(tc.tile_pool(name="const", bufs=1))
        persist = ctx.enter_context(tc.tile_pool(name="persist", bufs=1))
        expp = ctx.enter_context(tc.tile_pool(name="expp", bufs=4))
        rpool = ctx.enter_context(tc.tile_pool(name="rpool", bufs=4))
        otmp = ctx.enter_context(tc.tile_pool(name="otmp", bufs=2))
        outp = ctx.enter_context(tc.tile_pool(name="outp", bufs=2))

        # --- persistent feature-major activations ---
        qT_sb = persist.tile([128, CT, NQ], BF16)
        kT_sb = persist.tile([128, CT, NC], BF16)
        v_sb = persist.tile([128, JB, H * 65], BF16)  # [v_h | 1] per head
        oT_sb = persist.tile([128, CT, NQ], BF16)
        wo_sb = persist.tile([128, CT, DQ], BF16)
        bo_sb = const.tile([1, DQ], BF16)
        onesF = const.tile([128, 128], F32)
        nc.vector.memset(onesF, 1.0)
        ones1 = const.tile([1, 128], BF16)  # bias-matmul lhsT
        nc.vector.tensor_copy(ones1, onesF[0:1, :])

        v4 = v_sb.rearrange("p j (h e) -> p j h e", e=65)
        for jb in range(JB):
            nc.vector.tensor_copy(v4[:, jb, :, 64:65], onesF[:, 0:H].unsqueeze(-1))

        # --- phase A: q^T = (x @ Wq)^T, chunk-major with 4 open groups ---
        with tc.tile_pool(name="phA", bufs=1) as phA:
            xT_sb = phA.tile([128, AT, NQ], BF16)
            wq_sb = phA.tile([128, AT, INNER], BF16)
            wk_sb = phA.tile([128, BT, INNER], BF16)
            wv_sb = phA.tile([128, BT, INNER], BF16)
            xTr = xT.rearrange("(t p) i -> p t i", p=128)
            wqr = wq.rearrange("(t p) c -> p t c", p=128)
            wkr = wk.rearrange("(t p) c -> p t c", p=128)
            wvr = wv.rearrange("(t p) c -> p t c", p=128)
            ctxTr = ctxT.rearrange("(t p) j -> p t j", p=128)
            # consolidated, priority-ordered input DMAs
            nc.sync.dma_start(out=wq_sb[:, 0:2, :], in_=wqr[:, 0:2, :])
            nc.sync.dma_start(out=xT_sb[:, 0:2, :], in_=xTr[:, 0:2, :])
            nc.sync.dma_start(out=wq_sb[:, 2:8, :], in_=wqr[:, 2:8, :])
            nc.sync.dma_start(out=xT_sb[:, 2:5, :], in_=xTr[:, 2:5, :])
            nc.sync.dma_start(out=xT_sb[:, 5:8, :], in_=xTr[:, 5:8, :])
            nc.sync.dma_start(out=wk_sb, in_=wkr)
            nc.sync.dma_start(out=wv_sb, in_=wvr)

            with tc.tile_pool(name="ps_a", bufs=1, space="PSUM") as ps_a:
                psA = [ps_a.tile([128, NQ], F32, tag=f"pa{t}", name=f"psA{t}")
                       for t in range(CT)]
                for a in range(AT):
                    for t in range(CT):
                        for ch in range(2):
                            nc.tensor.matmul(
                                psA[t][:, ch * 512:(ch + 1) * 512],
                                lhsT=wq_sb[:, a, t * 128:(t + 1) * 128],
                                rhs=xT_sb[:, a, ch * 512:(ch + 1) * 512],
                                start=(a == 0),
                                stop=(a == AT - 1),
                            )
                for t in range(CT):
                    nc.vector.tensor_copy(qT_sb[:, t, :], psA[t])

            # --- phase B: k^T and v from streamed ctx^T quarters ---
            with tc.tile_pool(name="phBx", bufs=2) as phBx, \
                 tc.tile_pool(name="ps_b", bufs=2, space="PSUM") as ps_b:
                for jq in range(4):
                    cx = phBx.tile([128, BT, 512], BF16, tag="cx")
                    nc.sync.dma_start(
                        out=cx, in_=ctxTr[:, :, jq * 512:(jq + 1) * 512]
                    )
                    if jq == 0:
                        # output-projection operands: needed much later
                        wor = wo.rearrange("(t p) e -> p t e", p=128)
                        nc.sync.dma_start(out=wo_sb, in_=wor)
                        nc.sync.dma_start(out=bo_sb, in_=bo.unsqueeze(0))
                    for t in range(CT):
                        ps = ps_b.tile([128, 512], F32, tag="pb")
                        for b in range(BT):
                            nc.tensor.matmul(
                                ps,
                                lhsT=wk_sb[:, b, t * 128:(t + 1) * 128],
                                rhs=cx[:, b, :],
                                start=(b == 0),
                                stop=(b == BT - 1),
                            )
                        nc.vector.tensor_copy(
                            kT_sb[:, t, jq * 512:(jq + 1) * 512], ps
                        )
                    for q in range(4):
                        jb = jq * 4 + q
                        ps = ps_b.tile([128, 512], F32, tag="pb")
                        for b in range(BT):
                            nc.tensor.matmul(
                                ps,
                                lhsT=cx[:, b, q * 128:(q + 1) * 128],
                                rhs=wv_sb[:, b, :],
                                start=(b == 0),
                                stop=(b == BT - 1),
                            )
                        nc.vector.tensor_copy(
                            v4[:, jb, :, 0:64],
                            ps.rearrange("p (h d) -> p h d", d=DH),
                        )

        # --- attention: head pairs; exp on ACT is the wall; keep PE warm ---
        with tc.tile_pool(name="ps_s", bufs=1, space="PSUM") as ps_s, \
             tc.tile_pool(name="ps_pv", bufs=1, space="PSUM") as ps_pv, \
             tc.tile_pool(name="ps_j", bufs=1, space="PSUM") as ps_j:
            junk = ps_j.tile([128, 512], F32, name="junk")

            def filler(n=1):
                # throwaway matmuls to fill PE idle slices (HAM hold-at-k8)
                for _ in range(n):
                    nc.tensor.matmul(
                        junk,
                        lhsT=kT_sb[0:64, 0, 0:128],
                        rhs=qT_sb[0:64, 0, 0:512],
                        start=True,
                        stop=True,
                        tile_position=(0, 0),
                    )

            for qh in range(2):
                q0 = qh * 512
                for t in range(CT):
                    he = 2 * t
                    opv = [
                        ps_pv.tile([65, 512], F32, tag=f"pv{h}",
                                   name=f"opv{h}")
                        for h in range(2)
                    ]
                    es_tiles = []
                    for jbp in range(8):
                        ess = []
                        for hh, (po, tp) in enumerate(((0, (0, 0)), (64, (64, 0)))):
                            sp = ps_s.tile([128, 1024], F32, tag=f"s{hh}")
                            for jj in range(2):
                                jb = jbp * 2 + jj
                                nc.tensor.matmul(
                                    sp[:, jj * 512:(jj + 1) * 512],
                                    lhsT=kT_sb[po:po + 64, t,
                                               jb * 128:(jb + 1) * 128],
                                    rhs=qT_sb[po:po + 64, t, q0:q0 + 512],
                                    start=True,
                                    stop=True,
                                    tile_position=tp,
                                )
                            # PV of the PREVIOUS round overlaps this round's
                            # exp drain.
                            if jbp > 0:
                                prev_es = es_tiles[jbp - 1][hh]
                                for jj in range(2):
                                    jb = (jbp - 1) * 2 + jj
                                    nc.tensor.matmul(
                                        opv[hh][:, :],
                                        lhsT=v4[:, jb, he + hh, :],
                                        rhs=prev_es[:,
                                                    jj * 512:(jj + 1) * 512],
                                        start=(jbp == 1 and jj == 0),
                                        stop=False,
                                    )
                            filler(1)
                            es = expp.tile([128, 1024], BF16, tag=f"es{hh}")
                            nc.scalar.activation(
                                es, sp, mybir.ActivationFunctionType.Exp,
                                scale=SCALE,
                            )
                            ess.append(es)
                        es_tiles.append(ess)
                    # drain: PV for the last round (jbp=7)
                    for hh in range(2):
                        for jj in range(2):
                            jb = 7 * 2 + jj
                            nc.tensor.matmul(
                                opv[hh][:, :],
                                lhsT=v4[:, jb, he + hh, :],
                                rhs=es_tiles[7][hh][:,
                                                    jj * 512:(jj + 1) * 512],
                                start=False,
                                stop=(jj == 1),
                            )
                        filler(2)
                    # normalize the pair (odd head first: its result takes an
                    # extra SBUF->SBUF DMA hop to partitions 64-127).
                    for hh in (1, 0):
                        dn = rpool.tile([65, 512], F32, tag=f"dn{hh}")
                        nc.vector.tensor_copy(dn[64:65, :], opv[hh][64:65, :])
                        r0 = rpool.tile([1, 512], F32, tag=f"r0{hh}")
                        nc.sync.dma_start(out=r0, in_=dn[64:65, :])
                        rr = rpool.tile([1, 512], F32, tag=f"rr{hh}")
                        nc.vector.reciprocal_approx_fast(rr, r0)
                        rbx = rpool.tile([64, 512], F32, tag=f"rb{hh}")
                        nc.gpsimd.partition_broadcast(rbx, rr)
                        if hh == 0:
                            nc.vector.tensor_mul(
                                oT_sb[0:64, t, q0:q0 + 512],
                                opv[hh][0:64, :], rbx,
                            )
                        else:
                            ot = otmp.tile([64, 512], BF16, tag="ot")
                            nc.vector.tensor_mul(ot, opv[hh][0:64, :], rbx)
                            nc.sync.dma_start(
                                out=oT_sb[64:128, t, q0:q0 + 512], in_=ot
                            )

        # --- output projection: F = O^T.T @ Wo + bias ---
        with tc.tile_pool(name="ps_f", bufs=2, space="PSUM") as ps_f:
            for ib in range(IB):
                fp = ps_f.tile([128, NQ], F32, tag="pf")
                for ch in range(2):
                    for t in range(CT):
                        nc.tensor.matmul(
                            fp[:, ch * 512:(ch + 1) * 512],
                            lhsT=oT_sb[:, t, ib * 128:(ib + 1) * 128],
                            rhs=wo_sb[:, t, ch * 512:(ch + 1) * 512],
                            start=(t == 0),
                            stop=False,
                        )
                    nc.tensor.matmul(
                        fp[:, ch * 512:(ch + 1) * 512],
                        lhsT=ones1,
                        rhs=bo_sb[0:1, ch * 512:(ch + 1) * 512],
                        start=False,
                        stop=True,
                    )
                ost = outp.tile([128, DQ], F32)
                nc.vector.tensor_copy(ost, fp)
                nc.sync.dma_start(out=out[ib * 128:(ib + 1) * 128, :], in_=ost)


def get_program():
    if "nc" not in _CACHE:
        _CACHE["nc"] = _build_program()
    return _CACHE["nc"]


def make_in_maps(x, context, Wq, Wk, Wv, Wo, bo):
    bf = ml_dtypes.bfloat16
    in_maps = []
    wq_b = np.asarray(Wq).astype(bf)
    wk_b = np.asarray(Wk).astype(bf)
    wv_b = np.asarray(Wv).astype(bf)
    wo_b = np.asarray(Wo).astype(bf)
    bo_b = np.asarray(bo).astype(bf)
    for c in range(8):
        b, half = c // 2, c % 2
        in_maps.append({
            "xT": np.ascontiguousarray(
                x[b, half * NQ:(half + 1) * NQ, :].T
            ).astype(bf),
            "ctxT": np.ascontiguousarray(context[b].T).astype(bf),
            "Wq": wq_b,
            "Wk": wk_b,
            "Wv": wv_b,
            "Wo": wo_b,
            "bo": bo_b,
        })
    return in_maps


def kernel(x, context, Wq, Wk, Wv, Wo, bo):
    nc = get_program()
    in_maps = make_in_maps(x, context, Wq, Wk, Wv, Wo, bo)
    res = run_bass_kernel_spmd(nc, in_maps, list(range(8)))
    out = np.empty((B, NQ_FULL, DQ), np.float32)
    for c in range(8):
        b, half = c // 2, c % 2
        out[b, half * NQ:(half + 1) * NQ, :] = res.results[c]["out"]
    return out
